# revision 4
# baseline (speedup 1.0000x reference)
"""Trainium2 Bass kernel for nn_Diffusion_16758962389776.

Computes the mean BCE-with-logits loss between q_approx and the backward
diffusion posterior q(x_{t-1}=1 | x_t, x_0) over the strict lower triangle
of B=4 symmetric graphs of N=2048 nodes.

Math reduction
--------------
For a lower-tri element (i>j): a = adj_start[b,i,j] in {0,1},
x = (u[b,i,j] < thr(a)) with thr(a) = ft + a*(1-2*ft), ft = flip(t_b+1).
The BCE target is g[a,x] = lik1(x)*prior1(a)/ev(a,x), a 2x2 per-batch table.
loss = mean( softplus(q) - q*g[a,x] ).

Writing g[a,x] = n(a) + m(a)*x with n = C0 + C1*a, m = C2 + C3*a, the Markov
identity ft = fp + s*(1-2*fp) makes C3 == 0 for all t >= 1, so

  sum q*g = C2 * sum( q * (x + (C1/C2)*a + C0/C2) )

which is two fused scalar_tensor_tensor ops on the vector engine (the second
with a fused free-dim accumulation), after one is_lt compare.

Per core (one half of one batch's lower triangle, tril-linear layout):
  ACT: thr = Identity(a*c + ft); e = Exp(q); sp = Ln(e+1) with fused accum
  DVE: x = (u is_lt thr); w1 = D1*a + x; (w1 + D0)*q with fused accum
Host: gathers the per-core [128, 8] partial-sum tensors and finishes in f64.

Sharding: 8 cores = 4 batches x 2 halves. Host extracts the strict lower
triangle (the only data the reference reads) into contiguous per-core
[128, 8192] arrays, padded with 512 neutral elements (q=0 so only the
softplus sum is affected; the exact 512*ln(2) is subtracted on host).

Fallback: if any t == 0 (the Qt[-1] wraparound makes C3 != 0) the kernel is
rebuilt with ACT passes m = Identity(a*C3 + C2), n = Identity(a*C1 + C0)
and the DVE computes sum(q*(n + m*x)) without the C2 rescale.
"""

import math

import numpy as np

B = 4
N = 2048
E = N * (N - 1) // 2          # 2096128
TIMESTEPS = 1000
SPEED = 0.01
P = 128                       # SBUF partitions
W = 8192                      # free dim per core
PER_CORE = P * W              # 1048576
HALF = E // 2                 # 1048064 valid elements per core
NPAD = PER_CORE - HALF        # 512
F = 2048                      # tile free dim
NT = W // F                   # 4 tiles
NCORES = 8

_TRIL = None                  # cached (ti, tj)
_PROGRAMS = {}                # (use_m_pass, repeat) -> compiled Bacc


def _tril_indices():
    global _TRIL
    if _TRIL is None:
        _TRIL = np.tril_indices(N, -1)
    return _TRIL


def _flip32(k):
    """flip value of Qt[k-1], mimicking the reference's f32 arithmetic."""
    return np.float32(0.5) * (np.float32(1.0) - np.float32(0.98) ** np.float32(k))


def _batch_constants(tb):
    """Per-batch scalars (f64)."""
    ft = float(_flip32(tb + 1))                     # Qt[t] flip
    fp = float(_flip32(tb) if tb >= 1 else _flip32(TIMESTEPS))  # Qt[t-1] (wraps)
    f1 = float(_flip32(1))                          # Qt[0] flip
    g = np.zeros((2, 2), dtype=np.float64)
    for a in (0, 1):
        for x in (0, 1):
            lik1 = f1 + x * (1.0 - 2.0 * f1)
            prior1 = fp + a * (1.0 - 2.0 * fp)
            ev = (1.0 - ft) if a == x else ft
            g[a, x] = lik1 * prior1 / ev
    C0 = g[0, 0]
    C1 = g[1, 0] - g[0, 0]
    C2 = g[0, 1] - g[0, 0]
    C3 = g[1, 1] - g[1, 0] - g[0, 1] + g[0, 0]
    return dict(ft=ft, c=1.0 - 2.0 * ft, C0=C0, C1=C1, C2=C2, C3=C3,
                D0=C0 / C2, D1=C1 / C2)


def _build_program(use_m_pass, repeat=1):
    import concourse.bacc as bacc
    import concourse.mybir as mybir
    from concourse.mybir import AluOpType as op
    from concourse.tile import TileContext

    AF = mybir.ActivationFunctionType
    f32 = mybir.dt.float32
    i32 = mybir.dt.int32

    nc = bacc.Bacc("TRN2", target_bir_lowering=False, debug=False,
                   num_devices=NCORES)
    a_d = nc.dram_tensor("a_in", [P, W], i32, kind="ExternalInput").ap()
    u_d = nc.dram_tensor("u_in", [P, W], f32, kind="ExternalInput").ap()
    q_d = nc.dram_tensor("q_in", [P, W], f32, kind="ExternalInput").ap()
    c_d = nc.dram_tensor("cst", [P, 8], f32, kind="ExternalInput").ap()
    o_d = nc.dram_tensor("out", [P, 2 * NT], f32, kind="ExternalOutput").ap()

    with TileContext(nc) as tc:
        with tc.tile_pool(name="consts", bufs=1) as cpool, \
             tc.tile_pool(name="io", bufs=3) as io, \
             tc.tile_pool(name="scr", bufs=2) as scr, \
             tc.tile_pool(name="accs", bufs=1) as accp:
            cst = cpool.tile([P, 8], f32)
            nc.sync.dma_start(cst[:], c_d[:])
            ft_ap = cst[:, 0:1]
            c_ap = cst[:, 1:2]
            C1_ap = cst[:, 2:3]
            C2_ap = cst[:, 3:4]
            C3_ap = cst[:, 4:5]
            D0_ap = cst[:, 5:6]
            D1_ap = cst[:, 6:7]
            C0_ap = cst[:, 7:8]

            qwcol = accp.tile([P, NT], f32)
            spcol = accp.tile([P, NT], f32)

            for r in range(repeat):
                last = r == repeat - 1
                for t in range(NT):
                    sl = slice(t * F, (t + 1) * F)
                    a_t = io.tile([P, F], i32, tag="a", name=f"a_{r}_{t}")
                    u_t = io.tile([P, F], f32, tag="u", name=f"u_{r}_{t}")
                    q_t = io.tile([P, F], f32, tag="q", name=f"q_{r}_{t}")
                    nc.sync.dma_start(a_t[:], a_d[:, sl])
                    nc.sync.dma_start(u_t[:], u_d[:, sl])
                    nc.sync.dma_start(q_t[:], q_d[:, sl])

                    thr = scr.tile([P, F], f32, tag="thr", name=f"thr{r}_{t}")
                    nc.scalar.activation(thr[:], a_t[:], AF.Identity,
                                         bias=ft_ap, scale=c_ap)
                    x_t = scr.tile([P, F], f32, tag="x", name=f"x{r}_{t}")
                    nc.vector.tensor_tensor(x_t[:], u_t[:], thr[:], op.is_lt)

                    e_t = scr.tile([P, F], f32, tag="e", name=f"e{r}_{t}")
                    nc.scalar.activation(e_t[:], q_t[:], AF.Exp)
                    sp_t = scr.tile([P, F], f32, tag="sp", name=f"sp{r}_{t}")
                    nc.scalar.activation(
                        sp_t[:], e_t[:], AF.Ln, bias=1.0,
                        accum_out=spcol[:, t:t + 1] if last else None)

                    if use_m_pass:
                        m_t = scr.tile([P, F], f32, tag="m", name=f"m{r}_{t}")
                        nc.scalar.activation(m_t[:], a_t[:], AF.Identity,
                                             bias=C2_ap, scale=C3_ap)
                        n_t = scr.tile([P, F], f32, tag="n", name=f"n{r}_{t}")
                        nc.scalar.activation(n_t[:], a_t[:], AF.Identity,
                                             bias=C0_ap, scale=C1_ap)
                        w1 = scr.tile([P, F], f32, tag="w1", name=f"w1{r}_{t}")
                        nc.vector.tensor_tensor(w1[:], x_t[:], m_t[:], op.mult)
                        w2 = scr.tile([P, F], f32, tag="w2", name=f"w2{r}_{t}")
                        nc.vector.tensor_tensor(w2[:], w1[:], n_t[:], op.add)
                        j_t = scr.tile([P, F], f32, tag="j", name=f"j{r}_{t}")
                        nc.vector.scalar_tensor_tensor(
                            j_t[:], w2[:], 0.0, q_t[:], op.add, op.mult,
                            accum_out=qwcol[:, t:t + 1] if last else None)
                    else:
                        w1 = scr.tile([P, F], f32, tag="w1", name=f"w1{r}_{t}")
                        nc.vector.scalar_tensor_tensor(
                            w1[:], a_t[:], D1_ap, x_t[:], op.mult, op.add)
                        j_t = scr.tile([P, F], f32, tag="j", name=f"j{r}_{t}")
                        nc.vector.scalar_tensor_tensor(
                            j_t[:], w1[:], D0_ap, q_t[:], op.add, op.mult,
                            accum_out=qwcol[:, t:t + 1] if last else None)

            nc.sync.dma_start(o_d[:, 0:NT], qwcol[:])
            nc.sync.dma_start(o_d[:, NT:2 * NT], spcol[:])

    nc.compile()
    return nc


def _get_program(use_m_pass, repeat=1):
    key = (use_m_pass, repeat)
    if key not in _PROGRAMS:
        _PROGRAMS[key] = _build_program(use_m_pass, repeat)
    return _PROGRAMS[key]


def _make_cst(k, use_m_pass):
    if use_m_pass:
        # fallback layout: ft, c, C1, C2, C3, D0, D1, C0 -- the n-pass reads
        # bias from slot 7 (C0) via C0_ap in the builder (see below).
        row = [k["ft"], k["c"], k["C1"], k["C2"], k["C3"], k["D0"], k["D1"],
               k["C0"]]
    else:
        row = [k["ft"], k["c"], k["C1"], k["C2"], k["C3"], k["D0"], k["D1"],
               k["C0"]]
    return np.ascontiguousarray(
        np.broadcast_to(np.array(row, dtype=np.float32), (P, 8)))


def _prepare_in_maps(adj_start, t, u, q_approx, use_m_pass):
    ti, tj = _tril_indices()
    in_maps = []
    combine = []
    for b in range(B):
        tb = int(t[b])
        k = _batch_constants(tb)
        cst = _make_cst(k, use_m_pass)
        a_lin = np.ascontiguousarray(adj_start[b][ti, tj], dtype=np.int32)
        u_lin = np.ascontiguousarray(u[b][ti, tj], dtype=np.float32)
        q_lin = np.ascontiguousarray(q_approx[b], dtype=np.float32)
        for h in range(2):
            sl = slice(h * HALF, (h + 1) * HALF)
            a_pad = np.zeros(PER_CORE, dtype=np.int32)
            a_pad[:HALF] = a_lin[sl]
            u_pad = np.full(PER_CORE, 2.0, dtype=np.float32)
            u_pad[:HALF] = u_lin[sl]
            q_pad = np.zeros(PER_CORE, dtype=np.float32)
            q_pad[:HALF] = q_lin[sl]
            in_maps.append({
                "a_in": a_pad.reshape(P, W),
                "u_in": u_pad.reshape(P, W),
                "q_in": q_pad.reshape(P, W),
                "cst": cst,
            })
            combine.append(k)
    return in_maps, combine


def _combine(results, combine, use_m_pass):
    total = 0.0
    for r, k in zip(results, combine):
        out = np.asarray(r["out"], dtype=np.float64)
        s_qw = out[:, 0:NT].sum()
        s_sp = out[:, NT:2 * NT].sum()
        s_sp -= NPAD * math.log(2.0)  # padding contributes softplus(0)
        coupling = s_qw if use_m_pass else k["C2"] * s_qw
        total += s_sp - coupling
    return np.float32(total / (B * E))


def run(adj_start, t, u, q_approx, trace=False, repeat=1, trace_kwargs=None):
    """Full pipeline; returns (loss, BassKernelResults)."""
    from concourse import bass_utils

    adj_start = np.asarray(adj_start)
    t = np.asarray(t).astype(np.int64).ravel()
    u = np.asarray(u)
    q_approx = np.asarray(q_approx)
    assert adj_start.shape == (B, N, N) and u.shape == (B, N, N)
    assert q_approx.shape == (B, E) and t.shape == (B,)

    use_m_pass = bool((t == 0).any())
    nc = _get_program(use_m_pass, repeat)
    in_maps, combine = _prepare_in_maps(adj_start, t, u, q_approx, use_m_pass)
    kwargs = {}
    if trace:
        kwargs["trace"] = True
        if trace_kwargs:
            kwargs.update(trace_kwargs)
    res = bass_utils.run_bass_kernel_spmd(
        nc, in_maps, core_ids=list(range(NCORES)), **kwargs)
    loss = _combine(res.results, combine, use_m_pass)
    return loss, res


def kernel(adj_start, t, u, q_approx):
    loss, _ = run(adj_start, t, u, q_approx)
    return np.array(loss, dtype=np.float32)


# revision 5
# speedup vs baseline: 44767.0267x; 44767.0267x over previous
"""Trainium2 Bass kernel for nn_Diffusion_16758962389776.

Computes the mean BCE-with-logits loss between q_approx and the backward
diffusion posterior q(x_{t-1}=1 | x_t, x_0) over the strict lower triangle
of B=4 symmetric graphs of N=2048 nodes.

Math reduction
--------------
For a lower-tri element (i>j): a = adj_start[b,i,j] in {0,1},
x = (u[b,i,j] < thr(a)) with thr(a) = ft + a*(1-2*ft), ft = flip(t_b+1).
The BCE target is g[a,x] = lik1(x)*prior1(a)/ev(a,x), a 2x2 per-batch table.
loss = mean( softplus(q) - q*g[a,x] ).

Writing g[a,x] = n(a) + m(a)*x with n = C0 + C1*a, m = C2 + C3*a, the Markov
identity ft = fp + s*(1-2*fp) makes C3 == 0 for all t >= 1, so

  sum q*g = C2 * sum( q * (x + (C1/C2)*a + C0/C2) )

which is two fused scalar_tensor_tensor ops on the vector engine (the second
with a fused free-dim accumulation), after one is_lt compare.

Per core (one half of one batch's lower triangle, tril-linear layout):
  ACT: thr = Identity(a*c + ft); e = Exp(q); sp = Ln(e+1) with fused accum
  DVE: x = (u is_lt thr); w1 = D1*a + x; (w1 + D0)*q with fused accum
Host: gathers the per-core [128, 8] partial-sum tensors and finishes in f64.

Sharding: 8 cores = 4 batches x 2 halves. Host extracts the strict lower
triangle (the only data the reference reads) into contiguous per-core
[128, 8192] arrays, padded with 512 neutral elements (q=0 so only the
softplus sum is affected; the exact 512*ln(2) is subtracted on host).

Fallback: if any t == 0 (the Qt[-1] wraparound makes C3 != 0) the kernel is
rebuilt with ACT passes m = Identity(a*C3 + C2), n = Identity(a*C1 + C0)
and the DVE computes sum(q*(n + m*x)) without the C2 rescale.
"""

import math

import numpy as np

B = 4
N = 2048
E = N * (N - 1) // 2          # 2096128
TIMESTEPS = 1000
SPEED = 0.01
P = 128                       # SBUF partitions
W = 8192                      # free dim per core
PER_CORE = P * W              # 1048576
HALF = E // 2                 # 1048064 valid elements per core
NPAD = PER_CORE - HALF        # 512
F = 2048                      # tile free dim
NT = W // F                   # 4 tiles
NCORES = 8

_TRIL = None                  # cached (ti, tj)
_PROGRAMS = {}                # (use_m_pass, repeat) -> compiled Bacc


def _tril_indices():
    global _TRIL
    if _TRIL is None:
        _TRIL = np.tril_indices(N, -1)
    return _TRIL


def _flip32(k):
    """flip value of Qt[k-1], mimicking the reference's f32 arithmetic."""
    return np.float32(0.5) * (np.float32(1.0) - np.float32(0.98) ** np.float32(k))


def _batch_constants(tb):
    """Per-batch scalars (f64)."""
    ft = float(_flip32(tb + 1))                     # Qt[t] flip
    fp = float(_flip32(tb) if tb >= 1 else _flip32(TIMESTEPS))  # Qt[t-1] (wraps)
    f1 = float(_flip32(1))                          # Qt[0] flip
    g = np.zeros((2, 2), dtype=np.float64)
    for a in (0, 1):
        for x in (0, 1):
            lik1 = f1 + x * (1.0 - 2.0 * f1)
            prior1 = fp + a * (1.0 - 2.0 * fp)
            ev = (1.0 - ft) if a == x else ft
            g[a, x] = lik1 * prior1 / ev
    C0 = g[0, 0]
    C1 = g[1, 0] - g[0, 0]
    C2 = g[0, 1] - g[0, 0]
    C3 = g[1, 1] - g[1, 0] - g[0, 1] + g[0, 0]
    return dict(ft=ft, c=1.0 - 2.0 * ft, C0=C0, C1=C1, C2=C2, C3=C3,
                D0=C0 / C2, D1=C1 / C2)


def _build_program(use_m_pass, repeat=1):
    import concourse.bacc as bacc
    import concourse.mybir as mybir
    from concourse.mybir import AluOpType as op
    from concourse.tile import TileContext

    AF = mybir.ActivationFunctionType
    f32 = mybir.dt.float32
    i32 = mybir.dt.int32

    nc = bacc.Bacc("TRN2", target_bir_lowering=False, debug=False,
                   num_devices=NCORES)
    a_d = nc.dram_tensor("a_in", [P, W], i32, kind="ExternalInput").ap()
    u_d = nc.dram_tensor("u_in", [P, W], f32, kind="ExternalInput").ap()
    q_d = nc.dram_tensor("q_in", [P, W], f32, kind="ExternalInput").ap()
    c_d = nc.dram_tensor("cst", [P, 8], f32, kind="ExternalInput").ap()
    o_d = nc.dram_tensor("out", [P, 2 * NT], f32, kind="ExternalOutput").ap()

    with TileContext(nc) as tc:
        with tc.tile_pool(name="consts", bufs=1) as cpool, \
             tc.tile_pool(name="io", bufs=3) as io, \
             tc.tile_pool(name="scr", bufs=2) as scr, \
             tc.tile_pool(name="accs", bufs=1) as accp:
            cst = cpool.tile([P, 8], f32)
            nc.sync.dma_start(cst[:], c_d[:])
            ft_ap = cst[:, 0:1]
            c_ap = cst[:, 1:2]
            C1_ap = cst[:, 2:3]
            C2_ap = cst[:, 3:4]
            C3_ap = cst[:, 4:5]
            D0_ap = cst[:, 5:6]
            D1_ap = cst[:, 6:7]
            C0_ap = cst[:, 7:8]

            qwcol = accp.tile([P, NT], f32)
            spcol = accp.tile([P, NT], f32)

            for r in range(repeat):
                last = r == repeat - 1
                for t in range(NT):
                    sl = slice(t * F, (t + 1) * F)
                    a_t = io.tile([P, F], i32, tag="a", name=f"a_{r}_{t}")
                    u_t = io.tile([P, F], f32, tag="u", name=f"u_{r}_{t}")
                    q_t = io.tile([P, F], f32, tag="q", name=f"q_{r}_{t}")
                    nc.sync.dma_start(a_t[:], a_d[:, sl])
                    nc.sync.dma_start(u_t[:], u_d[:, sl])
                    nc.sync.dma_start(q_t[:], q_d[:, sl])

                    thr = scr.tile([P, F], f32, tag="thr", name=f"thr{r}_{t}")
                    nc.scalar.activation(thr[:], a_t[:], AF.Identity,
                                         bias=ft_ap, scale=c_ap)
                    x_t = scr.tile([P, F], f32, tag="x", name=f"x{r}_{t}")
                    nc.vector.tensor_tensor(x_t[:], u_t[:], thr[:], op.is_lt)

                    e_t = scr.tile([P, F], f32, tag="e", name=f"e{r}_{t}")
                    nc.scalar.activation(e_t[:], q_t[:], AF.Exp)
                    sp_t = scr.tile([P, F], f32, tag="sp", name=f"sp{r}_{t}")
                    nc.scalar.activation(
                        sp_t[:], e_t[:], AF.Ln, bias=1.0,
                        accum_out=spcol[:, t:t + 1] if last else None)

                    if use_m_pass:
                        m_t = scr.tile([P, F], f32, tag="m", name=f"m{r}_{t}")
                        nc.scalar.activation(m_t[:], a_t[:], AF.Identity,
                                             bias=C2_ap, scale=C3_ap)
                        n_t = scr.tile([P, F], f32, tag="n", name=f"n{r}_{t}")
                        nc.scalar.activation(n_t[:], a_t[:], AF.Identity,
                                             bias=C0_ap, scale=C1_ap)
                        w1 = scr.tile([P, F], f32, tag="w1", name=f"w1{r}_{t}")
                        nc.vector.tensor_tensor(w1[:], x_t[:], m_t[:], op.mult)
                        w2 = scr.tile([P, F], f32, tag="w2", name=f"w2{r}_{t}")
                        nc.vector.tensor_tensor(w2[:], w1[:], n_t[:], op.add)
                        j_t = scr.tile([P, F], f32, tag="j", name=f"j{r}_{t}")
                        nc.vector.scalar_tensor_tensor(
                            j_t[:], w2[:], 0.0, q_t[:], op.add, op.mult,
                            accum_out=qwcol[:, t:t + 1] if last else None)
                    else:
                        w1 = scr.tile([P, F], f32, tag="w1", name=f"w1{r}_{t}")
                        nc.vector.scalar_tensor_tensor(
                            w1[:], a_t[:], D1_ap, x_t[:], op.mult, op.add)
                        j_t = scr.tile([P, F], f32, tag="j", name=f"j{r}_{t}")
                        nc.vector.scalar_tensor_tensor(
                            j_t[:], w1[:], D0_ap, q_t[:], op.add, op.mult,
                            accum_out=qwcol[:, t:t + 1] if last else None)

            nc.sync.dma_start(o_d[:, 0:NT], qwcol[:])
            nc.sync.dma_start(o_d[:, NT:2 * NT], spcol[:])

    nc.compile()
    return nc


def _get_program(use_m_pass, repeat=1):
    key = (use_m_pass, repeat)
    if key not in _PROGRAMS:
        _PROGRAMS[key] = _build_program(use_m_pass, repeat)
    return _PROGRAMS[key]


def _make_cst(k, use_m_pass=False):
    # slots: ft, c, C1, C2, C3, D0, D1, C0 (broadcast to all partitions)
    row = [k["ft"], k["c"], k["C1"], k["C2"], k["C3"], k["D0"], k["D1"],
           k["C0"]]
    return np.ascontiguousarray(
        np.broadcast_to(np.array(row, dtype=np.float32), (P, 8)))


def _prepare_in_maps(adj_start, t, u, q_approx, use_m_pass):
    ti, tj = _tril_indices()
    in_maps = []
    combine = []
    for b in range(B):
        tb = int(t[b])
        k = _batch_constants(tb)
        cst = _make_cst(k, use_m_pass)
        a_lin = np.ascontiguousarray(adj_start[b][ti, tj], dtype=np.int32)
        u_lin = np.ascontiguousarray(u[b][ti, tj], dtype=np.float32)
        q_lin = np.ascontiguousarray(q_approx[b], dtype=np.float32)
        for h in range(2):
            sl = slice(h * HALF, (h + 1) * HALF)
            a_pad = np.zeros(PER_CORE, dtype=np.int32)
            a_pad[:HALF] = a_lin[sl]
            u_pad = np.full(PER_CORE, 2.0, dtype=np.float32)
            u_pad[:HALF] = u_lin[sl]
            q_pad = np.zeros(PER_CORE, dtype=np.float32)
            q_pad[:HALF] = q_lin[sl]
            in_maps.append({
                "a_in": a_pad.reshape(P, W),
                "u_in": u_pad.reshape(P, W),
                "q_in": q_pad.reshape(P, W),
                "cst": cst,
            })
            combine.append(k)
    return in_maps, combine


def _combine(results, combine, use_m_pass):
    total = 0.0
    for r, k in zip(results, combine):
        out = np.asarray(r["out"], dtype=np.float64)
        s_qw = out[:, 0:NT].sum()
        s_sp = out[:, NT:2 * NT].sum()
        s_sp -= NPAD * math.log(2.0)  # padding contributes softplus(0)
        coupling = s_qw if use_m_pass else k["C2"] * s_qw
        total += s_sp - coupling
    return np.float32(total / (B * E))


def run(adj_start, t, u, q_approx, trace=False, repeat=1, trace_kwargs=None):
    """Full pipeline; returns (loss, BassKernelResults)."""
    from concourse import bass_utils

    adj_start = np.asarray(adj_start)
    t = np.asarray(t).astype(np.int64).ravel()
    u = np.asarray(u)
    q_approx = np.asarray(q_approx)
    assert adj_start.shape == (B, N, N) and u.shape == (B, N, N)
    assert q_approx.shape == (B, E) and t.shape == (B,)

    use_m_pass = bool((t == 0).any())
    nc = _get_program(use_m_pass, repeat)
    in_maps, combine = _prepare_in_maps(adj_start, t, u, q_approx, use_m_pass)
    kwargs = {}
    if trace:
        kwargs["trace"] = True
        if trace_kwargs:
            kwargs.update(trace_kwargs)
    res = bass_utils.run_bass_kernel_spmd(
        nc, in_maps, core_ids=list(range(NCORES)), **kwargs)
    loss = _combine(res.results, combine, use_m_pass)
    return loss, res


def kernel(adj_start, t, u, q_approx):
    loss, _ = run(adj_start, t, u, q_approx)
    return np.array(loss, dtype=np.float32)
